# revision 1
# baseline (speedup 1.0000x reference)
"""Decision Transformer on 8 Trainium2 NeuronCores (Bass/Tile) — v2.

Sharding: data-parallel over batch (B=32 -> 4 seqs/core), no collectives.
Layout: feature-major activations [H, tokens]; bf16 weights + bf16 moving
operands everywhere (PE 1 cyc/row), f32 psum/residual. Elementwise work
split across DVE and the otherwise-idle Pool (gpsimd) engine; Activation
engine reserved for exp/gelu/sigmoid/tanh/LN-sqrt.
"""
import os
import numpy as np
import ml_dtypes

import concourse.bass as bass
import concourse.tile as tile
from concourse import bacc, mybir
from concourse.bass_utils import run_bass_kernel_spmd

AF = mybir.ActivationFunctionType
ALU = mybir.AluOpType
F32 = mybir.dt.float32
F32R = mybir.dt.float32r
BF16 = mybir.dt.bfloat16
F8 = mybir.dt.float8e4
DR = mybir.MatmulPerfMode.DoubleRow

# Model dims (hardcoded per contest spec)
H, L, NHEAD, V, MAXEP = 768, 12, 12, 1654, 50
B, S, TS, TA = 32, 50, 60, 8
T = 3 * S                    # 150 tokens per seq
NC_ = 8                      # cores
BC = B // NC_                # 4 seqs per core
NTOK = BC * T                # 600 transformer tokens per core
NG = BC * S                  # 200 GRU chains per core
HC = H // 128                # 6 feature chunks
G3 = 3 * H                   # 2304
GC = G3 // 128               # 18 gate chunks
FF = 4 * H                   # 3072
FC = FF // 128               # 24
HD = H // NHEAD              # 64
VP = 1656                    # padded vocab (4 x 414)
TOKC = [(0, 128), (128, 22)]  # per-seq token chunks of 150

# misc_f32 column offsets
MF_BHH = 0
MF_RETW = 18
MF_LNEG = 24
MF_LNEB = 30
MF_TE = 36
MF_TER = 1236
MF_WPE = 2436
MF_BIAS = 3336   # per layer 54 cols: ab 12 | pb 6 | fcb 24 | mpb 6 | avb 6
MF_ONES = 3984
MF_COLS = 4240
# misc_bf16 column offsets
MB_ONES = 0
MB_IDENT = 128
MB_MASK = 256    # per b: [kc0 150 | corner 22] x2 (pair-duplicated) = 344
MB_COLS = 1632

bf16 = ml_dtypes.bfloat16

_CACHED = {}


def _build():
    nc = bacc.Bacc("TRN2", target_bir_lowering=False, debug=False)
    D = {}

    def din(name, shape, dt):
        D[name] = nc.dram_tensor(name, shape, dt, kind="ExternalInput").ap()
        return D[name]

    # ---- DRAM inputs ----
    din("xg_s", [TS, 128, GC, NG], BF16)
    din("xg_a", [TA, 128, GC, NG], BF16)
    din("whh", [128, HC, G3], F8)
    din("attn_qk", [L, 6, 128, HC, 256], BF16)
    din("attn_wv", [L, 3, 128, HC, 256], BF16)
    din("attn_pw", [L, 3, 128, HC, 256], BF16)
    din("fc_w", [L, 12, 128, HC, 256], BF16)
    din("mlp_pw", [L, FF, H], BF16)
    din("head_w", [4, 4, 128, HC, 414], BF16)
    din("headb", [4, VP], F32)
    din("misc_f", [128, MF_COLS], F32)
    din("misc_b", [128, MB_COLS], BF16)
    din("rtg", [1, NG], F32)
    out = nc.dram_tensor("o", [4, NG, V], BF16, kind="ExternalOutput").ap()

    with tile.TileContext(nc) as tc:
        _body(tc, D, out)
    nc.compile()
    return nc


def _body(tc, D, out):
    nc = tc.nc
    from contextlib import ExitStack
    est = ExitStack()
    with est:
        persist = est.enter_context(tc.tile_pool(name="persist", bufs=1))
        sqp = est.enter_context(tc.tile_pool(name="sqp", bufs=1))
        bca = est.enter_context(tc.tile_pool(name="bca", bufs=1))

        # ---- persistent tiles ----
        miscf = persist.tile([128, MF_COLS], F32, tag="miscf")
        nc.sync.dma_start(miscf[:], D["misc_f"])
        miscb = persist.tile([128, MB_COLS], BF16, tag="miscb")
        nc.sync.dma_start(miscb[:], D["misc_b"])
        ones_r = persist.tile([128, 256], F32R, tag="ones_r")
        nc.sync.dma_start(ones_r[:],
                          D["misc_f"][:, MF_ONES:MF_ONES + 256].bitcast(F32R))
        bvec = est.enter_context(tc.tile_pool(name="bvec", bufs=2))
        prewh = est.enter_context(tc.tile_pool(name="prewh", bufs=1))

        ones_b = miscb[:, MB_ONES:MB_ONES + 128]
        ident = miscb[:, MB_IDENT:MB_IDENT + 128]
        bhh = miscf[:, MF_BHH:MF_BHH + GC]
        eps_t = persist.tile([128, 1], F32, tag="eps_t")
        nc.vector.memset(eps_t[:], 1e-5)
        x_resid = persist.tile([128, HC, NTOK], F32R, tag="x_resid")

        # =============== Phase A: GRUs + assembly ===============
        with tc.tile_pool(name="hp", bufs=1) as hp:
            rtg_bc = hp.tile([128, NG], F32, tag="rtg_bc")
            nc.gpsimd.dma_start(
                out=rtg_bc[:],
                in_=bass.AP(tensor=D["rtg"].tensor, offset=D["rtg"].offset,
                            ap=[[0, 128]] + D["rtg"].ap[1:]))
            hs = [hp.tile([128, HC, NG], BF16, tag=f"hs{i}", name=f"hs{i}")
                  for i in range(2)]
            ha = [hp.tile([128, HC, NG], BF16, tag=f"ha{i}", name=f"ha{i}")
                  for i in range(2)]
            h8s = [hp.tile([128, HC, NG], F8, tag=f"h8s{i}", name=f"h8s{i}")
                   for i in range(2)]
            h8a = [hp.tile([128, HC, NG], F8, tag=f"h8a{i}", name=f"h8a{i}")
                   for i in range(2)]
            nc.vector.memset(hs[0][:], 0.0)
            nc.gpsimd.memset(ha[0][:], 0.0)
            nc.vector.memset(h8s[0][:], 0.0)
            nc.gpsimd.memset(h8a[0][:], 0.0)

            with tc.tile_pool(name="gruw", bufs=1) as gruw, \
                 tc.tile_pool(name="xgp", bufs=2) as xgp, \
                 tc.tile_pool(name="gtmp", bufs=1) as gtmp, \
                 tc.tile_pool(name="gps", bufs=4, space="PSUM") as gps:

                whh = gruw.tile([128, HC, G3], F8, tag="whh")
                nc.sync.dma_start(whh[:], D["whh"])

                def gru_step(t, xg_dram, hpair, h8pair):
                    hcur, hnxt = hpair[t % 2], hpair[(t + 1) % 2]
                    h8cur, h8nxt = h8pair[t % 2], h8pair[(t + 1) % 2]
                    xg = xgp.tile([128, GC, NG], BF16, tag="xg", bufs=3)
                    nc.sync.dma_start(xg[:], xg_dram[t])
                    rzt = gtmp.tile([128, HC, 2, NG], BF16, tag="rzt",
                                    bufs=3)
                    for c in range(HC):  # r/z pair chunks
                        prz = gps.tile([128, 2, NG], F32, tag="rz", bufs=4)
                        for g, m in ((0, c), (1, HC + c)):
                            nc.tensor.matmul(prz[:, g, :], ident, xg[:, m, :],
                                             start=True, stop=(t == 0))
                            if t > 0:
                                for kd in range(3):
                                    nc.tensor.matmul(
                                        prz[:, g, :],
                                        whh[:, 2 * kd:2 * kd + 2,
                                            m * 128:(m + 1) * 128],
                                        h8cur[:, 2 * kd:2 * kd + 2, :],
                                        start=False, stop=(kd == 2),
                                        perf_mode=DR)
                        nc.scalar.activation(rzt[:, c, :, :], prz[:],
                                             AF.Sigmoid)
                    t1 = gtmp.tile([128, HC, NG], F32, tag="t1", bufs=3)
                    for c in range(HC):  # n-gate t1 per chunk (m = 12+c)
                        m = 12 + c
                        eng = nc.vector
                        if t > 0:
                            pn = gps.tile([128, NG], F32, tag="n", bufs=4)
                            for kd in range(3):
                                nc.tensor.matmul(
                                    pn[:], whh[:, 2 * kd:2 * kd + 2,
                                               m * 128:(m + 1) * 128],
                                    h8cur[:, 2 * kd:2 * kd + 2, :],
                                    start=(kd == 0), stop=(kd == 2),
                                    perf_mode=DR)
                            # t1 = (hn + bhh_n) * r
                            eng.scalar_tensor_tensor(
                                t1[:, c, :], pn[:], bhh[:, m:m + 1],
                                rzt[:, c, 0, :], ALU.add, ALU.mult)
                        else:
                            eng.tensor_scalar_mul(t1[:, c, :],
                                                  rzt[:, c, 0, :],
                                                  bhh[:, m:m + 1])
                    for i in range(3):  # pair-merged n/tanh/h-update
                        cs = slice(2 * i, 2 * i + 2)
                        t2 = gtmp.tile([128, 2, NG], F32, tag="t2", bufs=4)
                        nc.gpsimd.tensor_add(t2[:], t1[:, cs, :],
                                             xg[:, 12 + 2 * i:
                                                14 + 2 * i, :])
                        n_ = gtmp.tile([128, 2, NG], BF16, tag="n", bufs=4)
                        nc.scalar.activation(n_[:], t2[:], AF.Tanh)
                        d = gtmp.tile([128, 2, NG], BF16, tag="d", bufs=3)
                        nc.vector.tensor_sub(d[:], hcur[:, cs, :], n_[:])
                        e = gtmp.tile([128, 2, NG], BF16, tag="e", bufs=3)
                        nc.gpsimd.tensor_mul(e[:], rzt[:, cs, 1, :], d[:])
                        nc.gpsimd.tensor_add(hnxt[:, cs, :], n_[:], e[:])
                        nc.vector.tensor_add(h8nxt[:, cs, :], n_[:], e[:])

                for t in range(TS):
                    gru_step(t, D["xg_s"], hs, h8s)
                    if t < TA:
                        gru_step(t, D["xg_a"], ha, h8a)
            enc_s = hs[TS % 2]
            enc_a = ha[TA % 2]

            # ---- assemble x = interleave(R, s, a); ln_e; + wpe ----
            teT = miscf[:, MF_TE:MF_TE + HC * NG].rearrange(
                "p (c j) -> p c j", c=HC)
            terT = miscf[:, MF_TER:MF_TER + HC * NG].rearrange(
                "p (c j) -> p c j", c=HC)
            wpeT = miscf[:, MF_WPE:MF_WPE + HC * T].rearrange(
                "p (c t) -> p c t", c=HC)
            retw = miscf[:, MF_RETW:MF_RETW + HC]
            lneg = miscf[:, MF_LNEG:MF_LNEG + HC]
            lneb = miscf[:, MF_LNEB:MF_LNEB + HC]
            with tc.tile_pool(name="sst", bufs=1, space="PSUM") as sps:
                xv = x_resid[:].rearrange("p c (b s three) -> p c b three s",
                                          b=BC, three=3)
                for c in range(HC):
                    eng = nc.vector if c % 2 == 0 else nc.gpsimd
                    hsv = enc_s[:, c, :].rearrange("p (b s) -> p b s", b=BC)
                    hav = enc_a[:, c, :].rearrange("p (b s) -> p b s", b=BC)
                    tev = teT[:, c, :].rearrange("p (b s) -> p b s", b=BC)
                    trv = terT[:, c, :].rearrange("p (b s) -> p b s", b=BC)
                    rgv = rtg_bc[:].rearrange("p (b s) -> p b s", b=BC)
                    nc.vector.scalar_tensor_tensor(
                        xv[:, c, :, 0, :], rgv, retw[:, c:c + 1], trv,
                        ALU.mult, ALU.add)
                    eng.tensor_add(xv[:, c, :, 1, :], hsv, tev)
                    eng.tensor_add(xv[:, c, :, 2, :], hav, tev)
                mean, rs = _ln_stats(tc, nc, sps, bca, sqp, ones_r, x_resid,
                                     NTOK, eps_t)
                for c in range(HC):
                    tt = sqp.tile([128, NTOK], F32, tag="ln_t", bufs=4)
                    nc.vector.tensor_sub(tt[:], x_resid[:, c, :], mean[:])
                    nc.vector.tensor_mul(tt[:], tt[:], rs[:])
                    nc.scalar.activation(x_resid[:, c, :], tt[:], AF.Identity,
                                         bias=lneb[:, c:c + 1],
                                         scale=lneg[:, c:c + 1])
                    xb = x_resid[:, c, :].rearrange("p (b t) -> p b t", b=BC)
                    for b in range(BC):
                        eng = nc.vector if b % 2 == 0 else nc.gpsimd
                        eng.tensor_add(xb[:, b, :], xb[:, b, :], wpeT[:, c, :])

        # =============== Phase B: transformer layers ===============
        with tc.tile_pool(name="actB", bufs=1) as actB, \
             tc.tile_pool(name="wcp", bufs=1) as wcp, \
             tc.tile_pool(name="att_sb", bufs=1) as att_sb:
            for l in range(L):
                bsl = miscf[:, MF_BIAS + l * 54:MF_BIAS + (l + 1) * 54]
                ab = bsl[:, 0:12]
                pb = bsl[:, 12:18]
                fcb = bsl[:, 18:42]
                mpb = bsl[:, 42:48]
                avbp = bsl[:, 48:54]

                # ---- LN1 -> y (bf16) ----
                with tc.tile_pool(name="st1", bufs=1, space="PSUM") as sps:
                    mean, rs = _ln_stats(tc, nc, sps, bca, sqp, ones_r,
                                         x_resid, NTOK, eps_t)
                y = actB.tile([128, HC, NTOK], BF16, tag="y")
                for c in range(HC):
                    eng = nc.vector if c % 2 == 0 else nc.gpsimd
                    tt = sqp.tile([128, NTOK], F32, tag="ln_t", bufs=4)
                    eng.tensor_sub(tt[:], x_resid[:, c, :], mean[:])
                    eng.tensor_mul(y[:, c, :], tt[:], rs[:])

                # ---- qkv feature-major, then v transposed to token-major ----
                qk = actB.tile([128, 12, NTOK], BF16, tag="qk")
                vf = actB.tile([128, HC, NTOK], BF16, tag="vf")
                vtok = actB.tile([128, BC, 2, H], BF16, tag="vtok")
                with tc.tile_pool(name="mm1", bufs=1, space="PSUM") as mmp:
                    for mp in range(9):
                        wc = wcp.tile([128, HC, 256], BF16, tag="wc", bufs=4,
                                      name=f"wqk{l}_{mp}")
                        src = (D["attn_qk"][l, mp] if mp < 6
                               else D["attn_wv"][l, mp - 6])
                        nc.sync.dma_start(wc[:], src)
                        for g in range(2):
                            m = 2 * mp + g
                            for nh in range(2):
                                nsl = slice(nh * 300, nh * 300 + 300)
                                ps = mmp.tile([128, 300], F32, tag="mm",
                                              bufs=5)
                                for k in range(HC):
                                    nc.tensor.matmul(
                                        ps[:], wc[:, k, g * 128:(g + 1) * 128],
                                        y[:, k, nsl], start=(k == 0),
                                        stop=(k == HC - 1))
                                if m < 12:
                                    nc.scalar.activation(
                                        qk[:, m, nsl], ps[:], AF.Identity,
                                        bias=ab[:, m:m + 1])
                                else:
                                    nc.vector.tensor_scalar_add(
                                        vf[:, m - 12, nsl], ps[:],
                                        avbp[:, m - 12:m - 11])
                    # transpose vf -> vtok [tok, feat]
                    for b in range(BC):
                        for kc, (k0, ksz) in enumerate(TOKC):
                            pt = mmp.tile([128, HC, 128], BF16, tag="tp",
                                          bufs=2)
                            for c in range(HC):
                                nc.tensor.transpose(
                                    pt[:ksz, c, :],
                                    vf[:, c, b * T + k0:b * T + k0 + ksz],
                                    ident)
                            if kc == 0:
                                nc.vector.tensor_copy(vtok[:ksz, b, kc, :],
                                                      pt[:ksz, :, :])
                            else:
                                nc.scalar.copy(vtok[:ksz, b, kc, :],
                                               pt[:ksz, :, :])

                # ---- attention (bf16): s -> exp -> mask -> Z/AV -> scale ----
                ox = actB.tile([128, HC, NTOK], BF16, tag="ox")
                with tc.tile_pool(name="aps", bufs=1, space="PSUM") as aps:
                    for b in range(BC):
                        maskv = miscb[:, MB_MASK + b * 344:
                                      MB_MASK + (b + 1) * 344]
                        for hp_ in range(NHEAD // 2):
                            pss = aps.tile([128, 2, 172], F32, tag="s",
                                           bufs=4)
                            if os.environ.get("DT_SIMSAFE"):
                                nc.vector.memset(pss[22:, :, 150:172], 0.0)
                            for hs in range(2):
                                h = 2 * hp_ + hs
                                p0 = (h % 2) * 64
                                qh = qk[p0:p0 + 64, h // 2,
                                        b * T:(b + 1) * T]
                                kh = qk[p0:p0 + 64, 6 + h // 2,
                                        b * T:(b + 1) * T]
                                nc.tensor.matmul(pss[:, hs, 0:150],
                                                 kh[:, 0:128], qh[:],
                                                 start=True, stop=True)
                                nc.tensor.matmul(pss[:22, hs, 150:172],
                                                 kh[:, 128:150],
                                                 qh[:, 128:150],
                                                 start=True, stop=True)
                            au = att_sb.tile([128, 2, 172], BF16, tag="au",
                                             bufs=3)
                            nc.scalar.activation(
                                au[:].rearrange("p a c -> p (a c)"), pss[:],
                                AF.Exp, scale=0.125)
                            am = att_sb.tile([128, 2, 172], BF16, tag="am",
                                             bufs=3)
                            nc.gpsimd.tensor_mul(
                                am[:].rearrange("p a c -> p (a c)"),
                                au[:].rearrange("p a c -> p (a c)"), maskv)
                            zp = aps.tile([128, T], F32, tag="z", bufs=2)
                            op_ = aps.tile([128, T], F32, tag="o", bufs=2)
                            for hs in range(2):
                                h = 2 * hp_ + hs
                                p0 = (h % 2) * 64
                                psz = zp[p0:p0 + 64, :]
                                nc.tensor.matmul(psz[:, 0:128],
                                                 ones_b[:128, 0:64],
                                                 am[:, hs, 0:128],
                                                 start=True, stop=True)
                                nc.tensor.matmul(psz[:, 128:150],
                                                 ones_b[:128, 0:64],
                                                 am[:, hs, 128:150],
                                                 start=True, stop=False)
                                nc.tensor.matmul(psz[:, 128:150],
                                                 ones_b[:22, 0:64],
                                                 am[:22, hs, 150:172],
                                                 start=False, stop=True)
                                pso = op_[p0:p0 + 64, :]
                                nc.tensor.matmul(
                                    pso[:, 0:128],
                                    vtok[:128, b, 0, h * 64:(h + 1) * 64],
                                    am[:, hs, 0:128], start=True, stop=True)
                                nc.tensor.matmul(
                                    pso[:, 128:150],
                                    vtok[:128, b, 0, h * 64:(h + 1) * 64],
                                    am[:, hs, 128:150], start=True,
                                    stop=False)
                                nc.tensor.matmul(
                                    pso[:, 128:150],
                                    vtok[:22, b, 1, h * 64:(h + 1) * 64],
                                    am[:22, hs, 150:172], start=False,
                                    stop=True)
                            rzt = att_sb.tile([128, T], F32, tag="rz",
                                              bufs=2)
                            nc.vector.reciprocal(rzt[:], zp[:])
                            nc.vector.tensor_mul(
                                ox[:, hp_, b * T:(b + 1) * T], op_[:],
                                rzt[:])

                # ---- proj (bf16, column-streamed) + residual ----
                with tc.tile_pool(name="mm2", bufs=1, space="PSUM") as mmp:
                    for mp in range(3):
                        wcb = wcp.tile([128, HC, 256], BF16, tag="wcb",
                                       bufs=3, name=f"wpj{l}_{mp}")
                        nc.sync.dma_start(wcb[:], D["attn_pw"][l, mp])
                        for g in range(2):
                            m = 2 * mp + g
                            for nh in range(2):
                                nsl = slice(nh * 300, nh * 300 + 300)
                                ps = mmp.tile([128, 300], F32, tag="mm",
                                              bufs=6)
                                for k in range(HC):
                                    nc.tensor.matmul(
                                        ps[:],
                                        wcb[:, k, g * 128:(g + 1) * 128],
                                        ox[:, k, nsl], start=(k == 0),
                                        stop=(k == HC - 1))
                                nc.vector.scalar_tensor_tensor(
                                    x_resid[:, m, nsl], ps[:], pb[:, m:m + 1],
                                    x_resid[:, m, nsl], ALU.add, ALU.add)

                # ---- LN2 -> y2 (bf16) ----
                with tc.tile_pool(name="st2", bufs=1, space="PSUM") as sps:
                    mean2, rs2 = _ln_stats(tc, nc, sps, bca, sqp, ones_r,
                                           x_resid, NTOK, eps_t,
                                           psum_tag="st2")
                y2 = actB.tile([128, HC, NTOK], BF16, tag="y")
                for c in range(HC):
                    eng = nc.vector if c % 2 == 0 else nc.gpsimd
                    tt = sqp.tile([128, NTOK], F32, tag="ln_t", bufs=4)
                    eng.tensor_sub(tt[:], x_resid[:, c, :], mean2[:])
                    eng.tensor_mul(y2[:, c, :], tt[:], rs2[:])

                # ---- MLP: fc -> gelu -> proj ----
                gel = actB.tile([128, FC, NTOK], BF16, tag="gel")
                with tc.tile_pool(name="mm3", bufs=1, space="PSUM") as mmp:
                    for mp in range(12):
                        wc = wcp.tile([128, HC, 256], BF16, tag="wc", bufs=4,
                                      name=f"wfc{l}_{mp}")
                        nc.sync.dma_start(wc[:], D["fc_w"][l, mp])
                        for g in range(2):
                            m = 2 * mp + g
                            for nh in range(2):
                                nsl = slice(nh * 300, nh * 300 + 300)
                                ps = mmp.tile([128, 300], F32, tag="mm",
                                              bufs=6)
                                for k in range(HC):
                                    nc.tensor.matmul(
                                        ps[:], wc[:, k, g * 128:(g + 1) * 128],
                                        y2[:, k, nsl], start=(k == 0),
                                        stop=(k == HC - 1))
                                nc.scalar.activation(gel[:, m, nsl], ps[:],
                                                     AF.Gelu_apprx_tanh,
                                                     bias=fcb[:, m:m + 1])
                    mpw = D["mlp_pw"][l].rearrange("(k p) g -> p k g", p=128)
                    for nh in range(2):
                        nsl = slice(nh * 300, nh * 300 + 300)
                        pss = [mmp.tile([128, 300], F32, tag="mm", bufs=6,
                                        name=f"mpps{l}_{nh}_{i}")
                               for i in range(HC)]
                        for kq in range(6):
                            wmt = wcp.tile([128, 4, H], BF16, tag="wm",
                                           bufs=3, name=f"wml{l}_{nh}_{kq}")
                            nc.sync.dma_start(wmt[:],
                                              mpw[:, 4 * kq:4 * kq + 4, :])
                            for dk in range(4):
                                k = 4 * kq + dk
                                for m in range(HC):
                                    nc.tensor.matmul(
                                        pss[m][:],
                                        wmt[:, dk, m * 128:(m + 1) * 128],
                                        gel[:, k, nsl], start=(k == 0),
                                        stop=(k == FC - 1))
                        for m in range(HC):
                            nc.vector.scalar_tensor_tensor(
                                x_resid[:, m, nsl], pss[m][:],
                                mpb[:, m:m + 1], x_resid[:, m, nsl],
                                ALU.add, ALU.add)

        # =============== Phase C: lnf on state cols + heads ===============
        with tc.tile_pool(name="phC", bufs=1) as phC:
            x1v = x_resid[:].rearrange("p c (b s three) -> p c b three s",
                                       b=BC, three=3)
            x1 = phC.tile([128, HC, NG], BF16, tag="x1")
            with tc.tile_pool(name="hps", bufs=1, space="PSUM") as sps:
                psS = sps.tile([128, NG], F32, tag="hS")
                psQ = sps.tile([128, NG], F32, tag="hQ")
                for c in range(HC):
                    sq = sqp.tile([128, BC, S], F32R, tag="hsq", bufs=2)
                    nc.gpsimd.tensor_mul(sq[:], x1v[:, c, :, 1, :],
                                         x1v[:, c, :, 1, :])
                    nc.tensor.matmul(psS[:], ones_r[:, :128],
                                     x1v[:, c, :, 1, :],
                                     start=(c == 0), stop=(c == HC - 1))
                    nc.tensor.matmul(psQ[:], ones_r[:, :128], sq[:],
                                     start=(c == 0), stop=(c == HC - 1))
                mean = bca.tile([128, NG], F32, tag="hmean")
                rs = bca.tile([128, NG], F32, tag="hrs")
                nc.vector.tensor_scalar_mul(mean[:], psS[:], 1.0 / H)
                m2 = bca.tile([128, NG], F32, tag="hm2")
                nc.gpsimd.tensor_mul(m2[:], mean[:], mean[:])
                vv = bca.tile([128, NG], F32, tag="hvv")
                nc.vector.scalar_tensor_tensor(vv[:], psQ[:], 1.0 / H, m2[:],
                                               ALU.mult, ALU.subtract)
                sd = bca.tile([128, NG], F32, tag="hsd")
                nc.scalar.activation(sd[:], vv[:], AF.Sqrt, bias=eps_t[:])
                nc.vector.reciprocal(rs[:], sd[:])
                for c in range(HC):
                    eng = nc.vector if c % 2 == 0 else nc.gpsimd
                    tt = sqp.tile([128, NG], F32, tag="hln_t", bufs=4)
                    eng.tensor_sub(tt[:], x1v[:, c, :, 1, :], mean[:])
                    eng.tensor_mul(x1[:, c, :], tt[:], rs[:])

            with tc.tile_pool(name="ops", bufs=1, space="PSUM") as ops:
                for hd_ in range(4):
                    hb = phC.tile([1, VP], F32R, tag="hb", bufs=2)
                    nc.sync.dma_start(hb[:],
                                      D["headb"][hd_][None, :].bitcast(F32R))
                    for nv in range(4):
                        nv0 = nv * 414
                        nvsz = 414 if nv < 3 else 412
                        whc = prewh.tile([128, HC, 414], BF16, tag="wh",
                                         bufs=2, name=f"wh{hd_}_{nv}")
                        nc.sync.dma_start(whc[:], D["head_w"][hd_, nv])
                        for tci, (t0, tsz) in enumerate([(0, 128), (128, 72)]):
                            ot = phC.tile([128, 414], BF16, tag="ot",
                                          bufs=4, name=f"ot{hd_}_{nv}_{tci}")
                            ps = ops.tile([128, 414], F32, tag="hmm", bufs=4)
                            for k in range(HC):
                                nc.tensor.matmul(ps[:tsz, :],
                                                 x1[:, k, t0:t0 + tsz],
                                                 whc[:, k, :],
                                                 start=(k == 0), stop=False)
                            nc.tensor.matmul(ps[:tsz, :],
                                             ones_r[0:1, t0:t0 + tsz],
                                             hb[:, nv0:nv0 + 414],
                                             start=False, stop=True)
                            nc.scalar.activation(ot[:tsz, :],
                                                 ps[:tsz, :], AF.Tanh)
                            nc.sync.dma_start(
                                out[hd_, t0:t0 + tsz, nv0:nv0 + nvsz],
                                ot[:tsz, :nvsz])


def _ln_stats(tc, nc, sps, bca, sqp, ones_r, x_resid, ntok, eps_t,
              psum_tag="st"):
    """Mean/rstd over feature (partition) dim via all-ones matmuls.

    Returns broadcast tiles mean, rs of shape [128, ntok]."""
    nhalves = [(i * 300, min(300, ntok - i * 300))
               for i in range((ntok + 299) // 300)]
    psS = [sps.tile([128, nsz], F32, tag=f"{psum_tag}S{i}",
                    name=f"{psum_tag}S{i}", bufs=1)
           for i, (n0, nsz) in enumerate(nhalves)]
    psQ = [sps.tile([128, nsz], F32, tag=f"{psum_tag}Q{i}",
                    name=f"{psum_tag}Q{i}", bufs=1)
           for i, (n0, nsz) in enumerate(nhalves)]
    HCn = x_resid.shape[1]
    for c in range(HCn):
        sq = sqp.tile([128, ntok], F32R, tag="sq", bufs=2)
        nc.gpsimd.tensor_mul(sq[:], x_resid[:, c, :], x_resid[:, c, :])
        for i, (n0, nsz) in enumerate(nhalves):
            nc.tensor.matmul(psS[i][:], ones_r[:, :128],
                             x_resid[:, c, n0:n0 + nsz],
                             start=(c == 0), stop=(c == HCn - 1))
            nc.tensor.matmul(psQ[i][:], ones_r[:, :128], sq[:, n0:n0 + nsz],
                             start=(c == 0), stop=(c == HCn - 1))
    mean = bca.tile([128, ntok], F32, tag="mean")
    rs = bca.tile([128, ntok], F32, tag="rs")
    for i, (n0, nsz) in enumerate(nhalves):
        nsl = slice(n0, n0 + nsz)
        nc.vector.tensor_scalar_mul(mean[:, nsl], psS[i][:], 1.0 / H)
        m2 = bca.tile([128, 300], F32, tag="m2", bufs=2)
        nc.gpsimd.tensor_mul(m2[:, :nsz], mean[:, nsl], mean[:, nsl])
        vv = bca.tile([128, 300], F32, tag="vv", bufs=2)
        nc.vector.scalar_tensor_tensor(vv[:, :nsz], psQ[i][:], 1.0 / H,
                                       m2[:, :nsz], ALU.mult, ALU.subtract)
        sd = bca.tile([128, 300], F32, tag="sd", bufs=2)
        nc.scalar.activation(sd[:, :nsz], vv[:, :nsz], AF.Sqrt, bias=eps_t[:])
        nc.vector.reciprocal(rs[:, nsl], sd[:, :nsz])
    return mean, rs


# ====================== host side ======================

def _prep(inputs):
    """Host prep: per-core in_maps."""
    g = {k: np.asarray(v) for k, v in inputs.items()}
    f32 = np.float32

    word_emb = g["word_emb"].astype(f32)
    w_ih = g["gru_w_ih"].astype(f32)
    b_ih = g["gru_b_ih"].astype(f32)
    b_hh = g["gru_b_hh"].astype(f32)
    # vocab-sized input-transform table; fold b_hh for the r/z gates so
    # the sigmoid needs no per-gate bias (weights-only precompute)
    xg_table = word_emb @ w_ih.T + b_ih  # [V, 3H]
    xg_table[:, :2 * H] += b_hh[:2 * H]
    xg_table = xg_table.astype(bf16)

    te_full = g["time_emb"][g["timesteps"]]  # [B, S, H]
    ret_b = g["ret_b"].astype(f32)

    ln1_g, ln1_b = g["ln1_g"].astype(f32), g["ln1_b"].astype(f32)
    ln2_g, ln2_b = g["ln2_g"].astype(f32), g["ln2_b"].astype(f32)
    lnf_g, lnf_b = g["lnf_g"].astype(f32), g["lnf_b"].astype(f32)

    attn_wf = (g["attn_w"] * ln1_g[:, :, None]).astype(f32)
    attn_bf = (g["attn_b"]
               + np.einsum("lh,lhg->lg", ln1_b, g["attn_w"])).astype(f32)
    fc_wf = (g["fc_w"] * ln2_g[:, :, None]).astype(f32)
    fc_bf = (g["fc_b"]
             + np.einsum("lh,lhg->lg", ln2_b, g["fc_w"])).astype(f32)
    head_wf = (g["head_w"] * lnf_g[None, :, None]).astype(f32)
    head_bf = (g["head_b"]
               + np.einsum("h,khv->kv", lnf_b, g["head_w"])).astype(f32)

    # weight repacks (contiguous per-tile DMA layouts)
    attn_qk = np.ascontiguousarray(
        attn_wf[:, :, :2 * H].reshape(L, HC, 128, 6, 256)
        .transpose(0, 3, 2, 1, 4)).astype(bf16)
    attn_wv = np.ascontiguousarray(
        attn_wf[:, :, 2 * H:].reshape(L, HC, 128, 3, 256)
        .transpose(0, 3, 2, 1, 4)).astype(bf16)
    attn_pw = np.ascontiguousarray(
        g["attn_pw"].astype(f32).reshape(L, HC, 128, 3, 256)
        .transpose(0, 3, 2, 1, 4)).astype(bf16)
    fc_wT = np.ascontiguousarray(
        fc_wf.reshape(L, HC, 128, 12, 256).transpose(0, 3, 2, 1, 4)
    ).astype(bf16)
    mlp_pw = g["mlp_pw"].astype(bf16)
    head_pad = np.zeros((4, H, VP), f32)
    head_pad[:, :, :V] = head_wf
    head_wT = np.ascontiguousarray(
        head_pad.reshape(4, HC, 128, 4, 414).transpose(0, 3, 2, 1, 4)
    ).astype(bf16)
    headb_pad = np.zeros((4, VP), f32)
    headb_pad[:, :V] = head_bf
    whh_t = np.ascontiguousarray(
        g["gru_w_hh"].astype(f32).T.reshape(HC, 128, G3).transpose(1, 0, 2)
    ).astype(ml_dtypes.float8_e4m3fn)

    # misc_f32 (partition-major small data)
    def pmaj(vec, ncol):  # [ncol*128] -> [128, ncol]
        return vec.reshape(ncol, 128).T

    misc_shared = np.zeros((128, MF_COLS), f32)
    misc_shared[:, MF_BHH:MF_BHH + GC] = pmaj(b_hh, GC)
    misc_shared[:, MF_RETW:MF_RETW + HC] = pmaj(
        g["ret_w"].astype(f32).reshape(H), HC)
    misc_shared[:, MF_LNEG:MF_LNEG + HC] = pmaj(g["ln_e_g"].astype(f32), HC)
    misc_shared[:, MF_LNEB:MF_LNEB + HC] = pmaj(g["ln_e_b"].astype(f32), HC)
    wpeT = g["wpe"][:T].astype(f32).T.reshape(HC, 128, T).transpose(1, 0, 2)
    misc_shared[:, MF_WPE:MF_WPE + HC * T] = wpeT.reshape(128, HC * T)
    for l in range(L):
        o0 = MF_BIAS + l * 54
        misc_shared[:, o0:o0 + 12] = pmaj(attn_bf[l, :12 * 128], 12)
        misc_shared[:, o0 + 12:o0 + 18] = pmaj(g["attn_pb"][l].astype(f32),
                                               HC)
        misc_shared[:, o0 + 18:o0 + 42] = pmaj(fc_bf[l], FC)
        misc_shared[:, o0 + 42:o0 + 48] = pmaj(g["mlp_pb"][l].astype(f32),
                                               HC)
        misc_shared[:, o0 + 48:o0 + 54] = pmaj(attn_bf[l, 2 * H:], HC)
    misc_shared[:, MF_ONES:MF_ONES + 256] = 1.0

    # misc_bf16 (ones, ident, padded causal mask)
    tril = np.tril(np.ones((T, T), f32))  # [q, k]
    am_all = g["attention_mask"].astype(f32)

    shared = {
        "whh": whh_t, "attn_qk": attn_qk, "attn_wv": attn_wv,
        "attn_pw": attn_pw, "fc_w": fc_wT, "mlp_pw": mlp_pw,
        "head_w": head_wT, "headb": headb_pad,
    }

    in_maps = []
    for c in range(NC_):
        bs = slice(c * BC, (c + 1) * BC)
        st = g["states"][bs]          # [4, S, TS]
        ac = g["actions"][bs]
        xg_s = np.ascontiguousarray(
            xg_table[st].transpose(2, 3, 0, 1).reshape(TS, GC, 128, BC * S)
            .transpose(0, 2, 1, 3))
        xg_a = np.ascontiguousarray(
            xg_table[ac].transpose(2, 3, 0, 1).reshape(TA, GC, 128, BC * S)
            .transpose(0, 2, 1, 3))
        te = te_full[bs].astype(f32)  # [4, S, H]
        teT = te.transpose(2, 0, 1).reshape(HC, 128, NG).transpose(1, 0, 2)
        misc_f = misc_shared.copy()
        misc_f[:, MF_TE:MF_TE + HC * NG] = teT.reshape(128, HC * NG)
        misc_f[:, MF_TER:MF_TER + HC * NG] = (
            teT + pmaj(ret_b, HC)[:, :, None]).reshape(128, HC * NG)
        rtg = np.ascontiguousarray(
            g["returns_to_go"][bs, :, 0].astype(f32).reshape(1, NG))
        am3 = np.repeat(am_all[bs], 3, axis=1)  # [4, 150]
        maskT = (tril.T[None, :, :] * am3[:, :, None])  # [4, k, q]
        mask_tri = np.zeros((4, 128, 172), f32)
        mask_tri[:, :, 0:150] = maskT[:, 0:128, :]
        mask_tri[:, :22, 150:172] = maskT[:, 128:150, 128:150]
        mask_pair = np.concatenate([mask_tri, mask_tri], axis=2)
        misc_b = np.zeros((128, MB_COLS), bf16)
        misc_b[:, MB_ONES:MB_ONES + 128] = 1.0
        misc_b[:, MB_IDENT:MB_IDENT + 128] = np.eye(128, dtype=bf16)
        misc_b[:, MB_MASK:] = (
            mask_pair.transpose(1, 0, 2).reshape(128, 4 * 344).astype(bf16))
        m = dict(shared)
        m.update({"xg_s": xg_s, "xg_a": xg_a, "misc_f": misc_f,
                  "misc_b": misc_b, "rtg": rtg})
        in_maps.append(m)
    return in_maps


def kernel(**inputs):
    if "nc" not in _CACHED:
        _CACHED["nc"] = _build()
    nc = _CACHED["nc"]
    in_maps = _prep(inputs)
    res = run_bass_kernel_spmd(
        nc, in_maps, core_ids=list(range(NC_)),
        trace=bool(int(os.environ.get("DT_TRACE", "0"))))
    _CACHED["last"] = res
    outs = []
    for k in range(4):
        parts = [np.asarray(res.results[c]["o"][k], np.float32)
                 .reshape(BC, S, V) for c in range(NC_)]
        outs.append(np.concatenate(parts, axis=0))
    return tuple(outs)


def bench(inputs, iters=10):
    """Steady-state wall time of the jitted 8-core NEFF exec with
    device-resident inputs (ns). NTFF tracing is unavailable under this
    axon client, so this is the HW-time proxy."""
    import time
    import jax
    from jax.sharding import Mesh, PartitionSpec, NamedSharding
    from jax.experimental.shard_map import shard_map
    from concourse import mybir as _mb
    from concourse.bass2jax import (_bass_exec_p, install_neuronx_cc_hook,
                                    partition_id_tensor)

    if "nc" not in _CACHED:
        _CACHED["nc"] = _build()
    nc = _CACHED["nc"]
    in_maps = _prep(inputs)
    install_neuronx_cc_hook()

    in_names, out_names, out_avals, zero_shapes = [], [], [], []
    for alloc in nc.m.functions[0].allocations:
        if not isinstance(alloc, _mb.MemoryLocationSet):
            continue
        name = alloc.memorylocations[0].name
        pname = (nc.partition_id_tensor.name if nc.partition_id_tensor
                 else None)
        if alloc.kind == "ExternalInput":
            if name != pname:
                in_names.append(name)
        elif alloc.kind == "ExternalOutput":
            out_names.append(name)
            shape = tuple(alloc.tensor_shape)
            dtype = _mb.dt.np(alloc.dtype)
            out_avals.append(jax.core.ShapedArray(shape, dtype))
            zero_shapes.append((shape, dtype))
    n_params = len(in_names)
    n_outs = len(out_avals)
    all_names = in_names + out_names
    if nc.partition_id_tensor:
        all_names = all_names + [nc.partition_id_tensor.name]
    donate = tuple(range(n_params, n_params + n_outs))

    def _body(*args):
        operands = list(args)
        if nc.partition_id_tensor:
            operands.append(partition_id_tensor())
        return tuple(_bass_exec_p.bind(
            *operands, out_avals=tuple(out_avals), in_names=tuple(all_names),
            out_names=tuple(out_names), lowering_input_output_aliases=(),
            sim_require_finite=True, sim_require_nnan=True, nc=nc))

    devices = jax.devices()[:NC_]
    mesh = Mesh(np.asarray(devices), ("core",))
    spec = PartitionSpec("core")
    sharded = jax.jit(
        shard_map(_body, mesh=mesh, in_specs=(spec,) * (n_params + n_outs),
                  out_specs=(spec,) * n_outs, check_rep=False),
        donate_argnums=donate, keep_unused=True)

    sh = NamedSharding(mesh, spec)
    dev_in = [jax.device_put(
        np.concatenate([np.asarray(in_maps[c][n]) for c in range(NC_)],
                       axis=0), sh) for n in in_names]
    zeros_sets = [
        [jax.device_put(np.zeros((NC_ * s0[0], *s0[1:]), dt0), sh)
         for s0, dt0 in zero_shapes]
        for _ in range(iters + 1)
    ]
    # warmup (compiles)
    outs = sharded(*dev_in, *zeros_sets[0])
    jax.block_until_ready(outs)
    best = None
    times = []
    for i in range(iters):
        t0 = time.perf_counter()
        outs = sharded(*dev_in, *zeros_sets[i + 1])
        jax.block_until_ready(outs)
        dt = time.perf_counter() - t0
        times.append(dt)
        best = dt if best is None else min(best, dt)
    print("bench iters (ms):", [f"{t*1e3:.2f}" for t in times])
    _CACHED["bench_outs"] = [np.asarray(o) for o in outs]
    return int(best * 1e9)

